# revision 1
# baseline (speedup 1.0000x reference)
"""Trainium2 Bass kernel for nn_AttentionBlock (64, 512, 16) / three 8192x8192 Linears.

Strategy (8 NeuronCores, single NEFF, one launch):
  Phase 1 (QKV projection, column-sharded):
    Each core c owns output columns [1024c, 1024(c+1)) of each Linear
    (= w positions [64c, 64(c+1)), all 16 d). Weights are pre-transposed
    on host so every DMA tile is contiguous. q/k/v shards (64b x 1024c)
    accumulate in PSUM over 64 K-tiles of 128; bias is added via a K=1
    ones-matmul. q/k are permuted to [b][d][w_local] layout on the way
    out of PSUM (free-dim permute copy); v stays [b][w_local][d].
  AllToAll: core c sends batch-block j of its (3, 64b, 1024c) shard to
    core j. After A2A each core holds the full q/k/v for its own 8
    batches -- the attention program is fully static per core.
  Phase 2 (attention, batch-sharded, 8 batches/core):
    alphas are built transposed [k, q] so softmax over the *query* axis
    is a free-dim reduction (exp via ScalarE with fused accum row-sum);
    the softmax denominator is folded into v rows; the second einsum
    produces res^T [d, q]; sigmoid + residual add in [d, w] layout.
    The loop runs in two passes (exp-only, then sigmoid-only) so the
    ScalarE activation table is loaded exactly twice.
  Host: gathers per-core (8, 16, 512) outputs, transposes back.

DMA notes: HWDGE has two independent rings (SP=nc.sync, ACT=nc.scalar);
weight/bounce DMAs alternate between them, small gathers ride SWDGE
(nc.gpsimd). SBUF-side DMA access patterns are kept plain (partition
dim leading, no partition splits) -- fancy APs only on the DRAM side.
"""

import math

import numpy as np
import ml_dtypes

import concourse.bass as bass
import concourse.bacc as bacc
import concourse.mybir as mybir
import concourse.tile as tile
import concourse.bass_utils as bass_utils

N_CORES = 8
BS, W_DIM, D = 64, 512, 16
K = W_DIM * D            # 8192 contraction dim
CPC = K // N_CORES       # 1024 output cols per core
WPC = W_DIM // N_CORES   # 64 w positions per core
BPC = BS // N_CORES      # 8 batches per core
NKT = K // 128           # 64 k-tiles
SCALE = 1.0 / math.sqrt(K)

USE_BF16 = True          # compute dtype for matmul operands / A2A payload

_CACHE: dict = {}


def _build(use_bf16: bool, repeat: int = 1, skip_collective: bool = False,
           chunk: int | None = None, wbufs: int = 10):
    cdt = mybir.dt.bfloat16 if use_bf16 else mybir.dt.float32
    f32 = mybir.dt.float32
    if chunk is None:
        chunk = 1                         # k-tiles per W DMA (256KB bf16)
    nchunks = NKT // chunk

    nc = bacc.Bacc("TRN2", target_bir_lowering=False, debug=False,
                   num_devices=N_CORES)

    # xt is host-preswizzled to [128 p, 64 kt, 64 b] so the load is one
    # fully contiguous DMA.
    xt_d = nc.dram_tensor("xt", [128, NKT * BS], cdt, kind="ExternalInput")
    w_d = [nc.dram_tensor(n, [K, CPC], cdt, kind="ExternalInput")
           for n in ("wq", "wk", "wv")]
    b_d = [nc.dram_tensor(n, [1, CPC], cdt, kind="ExternalInput")
           for n in ("bq", "bk", "bv")]
    xtp_d = nc.dram_tensor("xtp", [BPC, D, W_DIM], f32, kind="ExternalInput")
    out_d = nc.dram_tensor("out", [BPC, D, W_DIM], f32, kind="ExternalOutput")

    hwdge = [nc.sync, nc.scalar]          # the two independent HWDGE rings
    wdma = [nc.sync, nc.scalar, nc.gpsimd]   # weight DMA issuers (3 paths)

    with tile.TileContext(nc) as tc:
        with (
            tc.tile_pool(name="constp", bufs=1) as constp,
            tc.tile_pool(name="sbp", bufs=1) as sbp,
            tc.tile_pool(name="dramp", bufs=1, space="DRAM") as dramp,
        ):
            xt_sb = constp.tile([128, NKT, BS], cdt)
            nc.sync.dma_start(
                xt_sb[:], xt_d[:, :].rearrange("p (kt b) -> p kt b", kt=NKT))
            ones = constp.tile([1, BS], cdt)
            nc.gpsimd.memset(ones[:], 1.0)
            b_sb = []
            for t in range(3):
                bt = constp.tile([1, CPC], cdt, name=f"bias{t}")
                nc.scalar.dma_start(bt[:], b_d[t][:, :])
                b_sb.append(bt)

            for _rep in range(repeat):
              with (
                  tc.tile_pool(name="wpa", bufs=wbufs) as wpa,
                  tc.tile_pool(name="wpb", bufs=wbufs) as wpb,
              ):
                qk_sb = sbp.tile([BS, 2 * CPC], cdt, name="qk_sb",
                                 tag="qk_sb")
                v_sb = sbp.tile([BS, CPC], cdt, name="v_sb", tag="v_sb")
                a2a1_in = dramp.tile([N_CORES, 2, BPC, CPC], cdt,
                                     tag="a2a1_in", name="a2a1_in")
                a2a1_out = dramp.tile([N_CORES, 2, BPC, CPC], cdt,
                                      tag="a2a1_out", name="a2a1_out")
                a2a2_in = dramp.tile([N_CORES, BPC, CPC], cdt,
                                     tag="a2a2_in", name="a2a2_in")
                a2a2_out = dramp.tile([N_CORES, BPC, CPC], cdt,
                                      tag="a2a2_out", name="a2a2_out")

                def w_chunk_matmuls(t, m, psum, wt):
                    for j in range(chunk):
                        kt = m * chunk + j
                        for h in range(2):
                            nc.tensor.matmul(
                                psum[:, h * 512:(h + 1) * 512],
                                xt_sb[:, kt, :],
                                wt[:, j, h * 512:(h + 1) * 512],
                                start=(kt == 0), stop=False)

                def bias_matmuls(t, psum):
                    for h in range(2):
                        nc.tensor.matmul(
                            psum[:, h * 512:(h + 1) * 512],
                            ones[:],
                            b_sb[t][:, h * 512:(h + 1) * 512],
                            start=False, stop=True)

                # ---- phase A: q,k projection ----
                with tc.tile_pool(name="qkps", bufs=1, space="PSUM") as qkps:
                    psA = [qkps.tile([BS, CPC], f32, name=f"ps{t}")
                           for t in range(2)]
                    for m in range(nchunks):
                        for t in range(2):
                            wt = wpa.tile([128, chunk, CPC], cdt,
                                          tag=f"w{t}", name=f"wt{t}")
                            hwdge[(m * 2 + t) % 2].dma_start(
                                wt[:],
                                w_d[t][:, :].rearrange(
                                    "(m j p) c -> m p j c", p=128,
                                    j=chunk)[m])
                            w_chunk_matmuls(t, m, psA[t], wt)
                    for t in range(2):
                        bias_matmuls(t, psA[t])
                        # permute cols (w d) -> (d w) while leaving PSUM
                        nc.vector.tensor_copy(
                            qk_sb[:, t * CPC:(t + 1) * CPC].rearrange(
                                "b (d w) -> b d w", w=WPC),
                            psA[t].rearrange("b (w d) -> b d w", d=D))

                for j in range(N_CORES):
                    hwdge[j % 2].dma_start(
                        a2a1_in[j].rearrange("t b c -> b t c"),
                        qk_sb[BPC * j:BPC * (j + 1), :].rearrange(
                            "b (t c) -> b t c", t=2))
                if skip_collective:
                    a2a1_out = a2a1_in
                else:
                    nc.gpsimd.collective_compute(
                        "AllToAll", mybir.AluOpType.bypass,
                        replica_groups=[list(range(N_CORES))],
                        ins=[a2a1_in.opt()], outs=[a2a1_out.opt()])

                # ---- phase B: v projection overlapped with attention
                # part A (alphas + exp + denom, which need only q,k) ----
                with (
                    tc.tile_pool(name="vps", bufs=1, space="PSUM") as vps,
                    tc.tile_pool(name="attps", bufs=3, space="PSUM") as attps,
                    tc.tile_pool(name="attp", bufs=3) as attp,
                    tc.tile_pool(name="keepp", bufs=1) as keepp,
                ):
                    expa_tiles = {}
                    rec_tiles = []

                    def emit_part_a(b):
                        qkT = attp.tile([D, 2, N_CORES, WPC], cdt,
                                        tag="qkT", name="qkT")
                        for t in range(2):
                            nc.gpsimd.dma_start(
                                qkT[:, t, :, :],
                                a2a1_out[:, t, b, :].rearrange(
                                    "i (d w) -> d i w", d=D))
                        den = attp.tile([128, 4], f32, tag="den", name="den")
                        # HW ACT accum_out accumulates (+=): zero first
                        nc.gpsimd.memset(den[:], 0.0)
                        for kw in range(4):
                            aT = attps.tile([128, 512], f32, tag="aT",
                                            name="aT")
                            nc.tensor.matmul(
                                aT[:], qkT[:, 1, 2 * kw:2 * kw + 2, :],
                                qkT[:, 0, :, :], start=True, stop=True)
                            ea = keepp.tile([128, 512], cdt,
                                            tag=f"ea{b}_{kw}",
                                            name=f"ea{b}_{kw}")
                            nc.scalar.activation(
                                ea[:], aT[:],
                                mybir.ActivationFunctionType.Exp,
                                scale=SCALE, accum_out=den[:, kw:kw + 1])
                            expa_tiles[(b, kw)] = ea
                        rec = keepp.tile([128, 4], f32, tag=f"rec{b}",
                                         name=f"rec{b}")
                        nc.vector.reciprocal(rec[:], den[:])
                        rec_tiles.append(rec)

                    psV = vps.tile([BS, CPC], f32, name="psv")
                    per = max(1, nchunks // BPC)
                    next_b = 0
                    for m in range(nchunks):
                        wt = wpb.tile([128, chunk, CPC], cdt, tag="w2",
                                      name="wt2")
                        hwdge[m % 2].dma_start(
                            wt[:],
                            w_d[2][:, :].rearrange(
                                "(m j p) c -> m p j c", p=128, j=chunk)[m])
                        w_chunk_matmuls(2, m, psV, wt)
                        while next_b < BPC and next_b <= m // per:
                            emit_part_a(next_b)
                            next_b += 1
                    while next_b < BPC:
                        emit_part_a(next_b)
                        next_b += 1
                    bias_matmuls(2, psV)
                    nc.vector.tensor_copy(v_sb[:], psV[:])

                    for j in range(N_CORES):
                        hwdge[j % 2].dma_start(
                            a2a2_in[j], v_sb[BPC * j:BPC * (j + 1), :])
                    if skip_collective:
                        a2a2_res = a2a2_in
                    else:
                        nc.gpsimd.collective_compute(
                            "AllToAll", mybir.AluOpType.bypass,
                            replica_groups=[list(range(N_CORES))],
                            ins=[a2a2_in.opt()], outs=[a2a2_out.opt()])
                        a2a2_res = a2a2_out
                    a2a_v = a2a2_res.rearrange("(kw h) b c -> kw h b c", h=2)

                    # ---- attention part B: fold denom into v, second
                    # einsum, then one sigmoid pass ----
                    rs_tiles = []
                    for b in range(BPC):
                        vt = attp.tile([128, 4, D], cdt, tag="vt", name="vt")
                        for half in range(2):
                            hwdge[half].dma_start(
                                vt[64 * half:64 * half + 64, :, :],
                                a2a_v[:, half, b, :].rearrange(
                                    "i (w d) -> w i d", d=D))
                        vs = attp.tile([128, 4, D], cdt, tag="vs", name="vs")
                        for kw in range(4):
                            nc.vector.tensor_scalar_mul(
                                vs[:, kw, :], vt[:, kw, :],
                                rec_tiles[b][:, kw:kw + 1])
                        rT = attps.tile([D, W_DIM], f32, tag="rT", name="rT")
                        for kw in range(4):
                            nc.tensor.matmul(
                                rT[:], vs[:, kw, :],
                                expa_tiles[(b, kw)][:],
                                start=(kw == 0), stop=(kw == 3))
                        rs = attp.tile([D, W_DIM], f32, tag=f"rs{b}",
                                       name=f"rs{b}", bufs=1)
                        nc.vector.tensor_copy(rs[:], rT[:])
                        rs_tiles.append(rs)
                    for b in range(BPC):
                        xb = attp.tile([D, W_DIM], f32, tag="xb", name="xb")
                        nc.scalar.dma_start(xb[:], xtp_d[b])
                        sg = attp.tile([D, W_DIM], f32, tag="sg", name="sg")
                        nc.scalar.activation(
                            sg[:], rs_tiles[b][:],
                            mybir.ActivationFunctionType.Sigmoid)
                        oo = attp.tile([D, W_DIM], f32, tag="oo", name="oo")
                        nc.vector.tensor_add(oo[:], sg[:], xb[:])
                        nc.sync.dma_start(out_d[b], oo[:])

    nc.compile()
    return nc


def _prep_in_maps(x_in, Wq, bq, Wk, bk, Wv, bv, use_bf16: bool):
    npdt = ml_dtypes.bfloat16 if use_bf16 else np.float32
    x_flat = np.ascontiguousarray(np.asarray(x_in, np.float32).reshape(BS, K))
    # swizzled x^T: [128 p, kt, b] contiguous
    xt = np.ascontiguousarray(
        x_flat.T.reshape(NKT, 128, BS).transpose(1, 0, 2)
    ).reshape(128, NKT * BS).astype(npdt)
    ws = [np.ascontiguousarray(np.asarray(W, np.float32).T).astype(npdt)
          for W in (Wq, Wk, Wv)]
    bs = [np.asarray(b, np.float32).reshape(1, K).astype(npdt)
          for b in (bq, bk, bv)]
    xtp = np.ascontiguousarray(
        np.asarray(x_in, np.float32).transpose(0, 2, 1))       # (BS, D, W)

    in_maps = []
    for c in range(N_CORES):
        cs = slice(CPC * c, CPC * (c + 1))
        m = {
            "xt": xt,
            "wq": np.ascontiguousarray(ws[0][:, cs]),
            "wk": np.ascontiguousarray(ws[1][:, cs]),
            "wv": np.ascontiguousarray(ws[2][:, cs]),
            "bq": np.ascontiguousarray(bs[0][:, cs]),
            "bk": np.ascontiguousarray(bs[1][:, cs]),
            "bv": np.ascontiguousarray(bs[2][:, cs]),
            "xtp": np.ascontiguousarray(xtp[BPC * c:BPC * (c + 1)]),
        }
        in_maps.append(m)
    return in_maps


def _assemble(results):
    out = np.empty((BS, W_DIM, D), np.float32)
    for c in range(N_CORES):
        o = results[c]["out"]                                   # (BPC, D, W)
        out[BPC * c:BPC * (c + 1)] = o.transpose(0, 2, 1)
    return out


def get_nc(use_bf16: bool = USE_BF16):
    key = ("nc", use_bf16)
    if key not in _CACHE:
        _CACHE[key] = _build(use_bf16)
    return _CACHE[key]


def kernel(x_in, Wq, bq, Wk, bk, Wv, bv):
    use_bf16 = USE_BF16
    nc = get_nc(use_bf16)
    in_maps = _prep_in_maps(x_in, Wq, bq, Wk, bk, Wv, bv, use_bf16)
    res = bass_utils.run_bass_kernel_spmd(
        nc, in_maps, core_ids=list(range(N_CORES)))
    return _assemble(res.results)



# revision 3
# speedup vs baseline: 1.5226x; 1.5226x over previous
"""Trainium2 Bass kernel for nn_AttentionBlock (64, 512, 16) / three 8192x8192 Linears.

v2 strategy (8 NeuronCores, single NEFF, one launch):
  fp8(e4m3) weights AND activations for the projection: W' = fp8(W^T*64),
  x' = fp8(x*16). The PE runs W-stationary matmuls (stationary [128,128]
  weight chunk, moving x' [128,64]) so the full 128-wide array is used:
  ~2x fewer PE-cycles than the x-stationary baseline, and fp8 halves the
  HBM weight traffic to ~25MB/core (~70us roofline at 358GB/s).

  Projection output lands transposed in PSUM ([cols, batch]); a PE
  transpose (identity matmul) flips each [128,64] chunk back to
  [batch, cols], with the 1/128 dequant scale applied in the DVE
  psum->sbuf copy (leaving q,k,v scaled by 8 -- folded into the exp and
  sigmoid activation scales downstream).

  Streaming order Wq, Wk (both HWDGE rings) then Wv (sync ring only, so
  ScalarE's queue stays free for exp). After qk: fp8 AllToAll #1; the
  attention part A (alphas + exp + DVE row-sum denominators) for the
  first 4 batches is interleaved into the tail of the Wv chunk loop so
  it fills PE/ScalarE gaps while Wv streams. After v: fp8 AllToAll #2,
  then part B (fold 1/den into v rows, second einsum, sigmoid+residual).

  Engine queues are FIFO: every instruction is emitted so nothing ever
  head-of-line-blocks a consumer that could run earlier (weight DMAs on
  sync/scalar only, gathers+collective triggers on gpsimd, exp/sigmoid
  on scalar emitted in dependency order).
"""

import math

import numpy as np
import ml_dtypes

import concourse.bass as bass
import concourse.bacc as bacc
import concourse.mybir as mybir
import concourse.tile as tile
import concourse.bass_utils as bass_utils

N_CORES = 8
BS, W_DIM, D = 64, 512, 16
K = W_DIM * D            # 8192 contraction dim
CPC = K // N_CORES       # 1024 output cols per core
WPC = W_DIM // N_CORES   # 64 w positions per core
BPC = BS // N_CORES      # 8 batches per core
NKT = K // 128           # 64 k-tiles
CH = 8                   # k-tiles per weight DMA chunk (1MB fp8)
NCH = NKT // CH

SX = 16.0                # host scale on x before fp8
SW = 64.0                # host scale on W before fp8
EPI_SCALE = 1.0 / 128.0  # psum->sbuf dequant; leaves q/k/v scaled by 8
ACT_S = SX * SW * EPI_SCALE                     # = 8
SCALE_EXP = (1.0 / math.sqrt(K)) / (ACT_S * ACT_S)
SIG_SCALE = 1.0 / ACT_S

_CACHE: dict = {}


def _build():
    f32 = mybir.dt.float32
    bf16 = mybir.dt.bfloat16
    f8 = mybir.dt.float8e4

    nc = bacc.Bacc("TRN2", target_bir_lowering=False, debug=False,
                   num_devices=N_CORES)

    # x' swizzled to [128 p, kt, b], fp8
    xq_d = nc.dram_tensor("xq", [128, NKT * BS], f8, kind="ExternalInput")
    w_d = [nc.dram_tensor(n, [K, CPC], f8, kind="ExternalInput")
           for n in ("wq", "wk", "wv")]
    b_d = [nc.dram_tensor(n, [1, CPC], bf16, kind="ExternalInput")
           for n in ("bq", "bk", "bv")]
    id_d = nc.dram_tensor("ident", [128, 128], bf16, kind="ExternalInput")
    xtp_d = nc.dram_tensor("xtp", [BPC, D, W_DIM], f32, kind="ExternalInput")
    out_d = nc.dram_tensor("out", [BPC, D, W_DIM], f32, kind="ExternalOutput")

    with tile.TileContext(nc) as tc:
        with (
            tc.tile_pool(name="constp", bufs=1) as constp,
            tc.tile_pool(name="sbp", bufs=1) as sbp,
            tc.tile_pool(name="dramp", bufs=1, space="DRAM") as dramp,
            tc.tile_pool(name="wp", bufs=3) as wp,
            tc.tile_pool(name="epi", bufs=2) as epi,
            tc.tile_pool(name="attp", bufs=3) as attp,
            tc.tile_pool(name="keepp", bufs=1) as keepp,
            tc.tile_pool(name="accp", bufs=2, space="PSUM") as accp,
            tc.tile_pool(name="tps", bufs=1, space="PSUM") as tps,
            tc.tile_pool(name="attps", bufs=3, space="PSUM") as attps,
            tc.tile_pool(name="rtps", bufs=2, space="PSUM") as rtps,
        ):
            # ---- constants ----
            xq_sb = constp.tile([128, NKT, BS], f8)
            nc.sync.dma_start(
                xq_sb[:], xq_d[:, :].rearrange("p (kt b) -> p kt b", kt=NKT))
            ident = constp.tile([128, 128], bf16)
            nc.scalar.dma_start(ident[:], id_d[:, :])
            ones = constp.tile([1, BS], bf16)
            nc.gpsimd.memset(ones[:], 1.0)
            b_sb = []
            for t in range(3):
                bt = constp.tile([1, CPC], bf16, name=f"bias{t}")
                nc.scalar.dma_start(bt[:], b_d[t][:, :])
                b_sb.append(bt)

            # ---- persistent activations ----
            qk_sb = sbp.tile([BS, 2, D, WPC], f8, name="qk_sb")
            v_sb = sbp.tile([BS, CPC], f8, name="v_sb")
            a2a1_in = dramp.tile([N_CORES, 2, BPC, CPC], f8,
                                 tag="a2a1_in", name="a2a1_in")
            a2a1_out = dramp.tile([N_CORES, 2, BPC, CPC], f8,
                                  tag="a2a1_out", name="a2a1_out")
            a2a2_in = dramp.tile([N_CORES, BPC, CPC], f8,
                                 tag="a2a2_in", name="a2a2_in")
            a2a2_out = dramp.tile([N_CORES, BPC, CPC], f8,
                                  tag="a2a2_out", name="a2a2_out")

            def mm_chunk(t, acc, m, wt):
                for j in range(CH):
                    kt = m * CH + j
                    for cc in range(8):
                        nc.tensor.matmul(
                            acc[:, cc, :],
                            wt[:, j, cc * 128:(cc + 1) * 128],
                            xq_sb[:, kt, :],
                            start=(kt == 0), stop=False)

            def bias_mms(t, acc):
                for cc in range(8):
                    nc.tensor.matmul(
                        acc[:, cc, :],
                        b_sb[t][:, cc * 128:(cc + 1) * 128],
                        ones[:], start=False, stop=True)

            def epilogue(t, acc):
                # psum [c,b] -> sbuf bf16 with dequant scale
                sb = epi.tile([128, 8, BS], bf16, tag="episb", name=f"esb{t}")
                nc.vector.tensor_scalar_mul(sb[:], acc[:], EPI_SCALE)
                ps = tps.tile([BS, CPC], bf16, tag="tpsum", name=f"tps{t}")
                for cc in range(8):
                    nc.tensor.transpose(
                        ps[:, cc * 128:(cc + 1) * 128], sb[:, cc, :],
                        ident[:])
                if t < 2:
                    # c index is (w d); store [b, d, w] for the a2a payload
                    nc.vector.tensor_copy(
                        qk_sb[:, t, :, :],
                        ps[:, :].rearrange("b (w d) -> b d w", d=D))
                else:
                    nc.vector.tensor_copy(v_sb[:], ps[:, :])

            # ================ phase 1: q then k ================
            for t in range(2):
                acc = accp.tile([128, 8, BS], f32, tag="acc", name=f"acc{t}")
                rings = [nc.sync, nc.scalar] if t == 0 else [nc.scalar, nc.sync]
                for m in range(NCH):
                    wt = wp.tile([128, CH, CPC], f8, tag="w", name=f"wt{t}")
                    rings[m % 2].dma_start(
                        wt[:],
                        w_d[t][:, :].rearrange(
                            "(m j p) c -> m p j c", p=128, j=CH)[m])
                    mm_chunk(t, acc, m, wt)
                bias_mms(t, acc)
                epilogue(t, acc)

            # a2a1 payload: [j, t, b, c] <- qk_sb[b, t, (d w)]
            for j in range(N_CORES):
                nc.scalar.dma_start(
                    a2a1_in[j].rearrange("t b c -> b t c"),
                    qk_sb[BPC * j:BPC * (j + 1)].rearrange(
                        "b t d w -> b t (d w)"))
            nc.gpsimd.collective_compute(
                "AllToAll", mybir.AluOpType.bypass,
                replica_groups=[list(range(N_CORES))],
                ins=[a2a1_in.opt()], outs=[a2a1_out.opt()])

            # ================ part A (per batch) ================
            ea_tiles = {}
            rec_tiles = []

            def emit_part_a(b):
                qkT = attp.tile([D, 2, N_CORES, WPC], f8, tag="qkT",
                                name="qkT")
                for t2 in range(2):
                    nc.gpsimd.dma_start(
                        qkT[:, t2, :, :],
                        a2a1_out[:, t2, b, :].rearrange(
                            "i (d w) -> d i w", d=D))
                den = keepp.tile([128, 4], f32, tag=f"den{b}", name=f"den{b}")
                for kw in range(4):
                    aT = attps.tile([128, 512], f32, tag="aT", name="aT")
                    nc.tensor.matmul(
                        aT[:], qkT[:, 1, 2 * kw:2 * kw + 2, :],
                        qkT[:, 0, :, :], start=True, stop=True)
                    ea = keepp.tile([128, 512], bf16, tag=f"ea{b}_{kw}",
                                    name=f"ea{b}_{kw}")
                    nc.scalar.activation(
                        ea[:], aT[:], mybir.ActivationFunctionType.Exp,
                        scale=SCALE_EXP)
                    nc.vector.tensor_reduce(
                        den[:, kw:kw + 1], ea[:],
                        axis=mybir.AxisListType.X, op=mybir.AluOpType.add)
                    ea_tiles[(b, kw)] = ea
                rec = keepp.tile([128, 4], f32, tag=f"rec{b}", name=f"rec{b}")
                nc.vector.reciprocal(rec[:], den[:])
                rec_tiles.append(rec)

            # ================ phase 2: v (sync ring only), part A
            # for b=0..3 interleaved into the chunk tail ================
            accv = accp.tile([128, 8, BS], f32, tag="acc", name="accv")
            for m in range(NCH):
                wt = wp.tile([128, CH, CPC], f8, tag="w", name="wtv")
                nc.sync.dma_start(
                    wt[:],
                    w_d[2][:, :].rearrange(
                        "(m j p) c -> m p j c", p=128, j=CH)[m])
                mm_chunk(2, accv, m, wt)
                if m >= NCH - 4:
                    emit_part_a(m - (NCH - 4))
            bias_mms(2, accv)
            epilogue(2, accv)
            for b in range(4, BPC):
                emit_part_a(b)

            # a2a2 payload + collective
            for j in range(N_CORES):
                nc.sync.dma_start(a2a2_in[j], v_sb[BPC * j:BPC * (j + 1), :])
            # residual input, one gather (gpsimd is idle here)
            xb_all = keepp.tile([D, BPC, W_DIM], f32, name="xb_all")
            nc.gpsimd.dma_start(
                xb_all[:], xtp_d[:, :, :].rearrange("b d w -> d b w"))
            nc.gpsimd.collective_compute(
                "AllToAll", mybir.AluOpType.bypass,
                replica_groups=[list(range(N_CORES))],
                ins=[a2a2_in.opt()], outs=[a2a2_out.opt()])

            # ================ part B ================
            # v rows for all batches: [h*64+w, b, kw, d], one DMA per (h,kw)
            vt_all = keepp.tile([128, BPC, 4, D], f8, name="vt_all")
            for kw in range(4):
                for h in range(2):
                    nc.sync.dma_start(
                        vt_all[64 * h:64 * h + 64, :, kw, :],
                        a2a2_out[2 * kw + h].rearrange(
                            "b (w d) -> w b d", d=D))
            for b in range(BPC):
                vs = attp.tile([128, 4, D], bf16, tag="vs", name="vs")
                for kw in range(4):
                    for h in range(2):
                        nc.vector.tensor_scalar_mul(
                            vs[64 * h:64 * h + 64, kw, :],
                            vt_all[64 * h:64 * h + 64, b, kw, :],
                            rec_tiles[b][64 * h:64 * h + 64, kw:kw + 1])
                rT = rtps.tile([D, W_DIM], f32, tag="rT", name="rT")
                for kw in range(4):
                    nc.tensor.matmul(
                        rT[:], vs[:, kw, :], ea_tiles[(b, kw)][:],
                        start=(kw == 0), stop=(kw == 3))
                sg = attp.tile([D, W_DIM], f32, tag="sg", name="sg")
                nc.scalar.activation(
                    sg[:], rT[:], mybir.ActivationFunctionType.Sigmoid,
                    scale=SIG_SCALE)
                oo = attp.tile([D, W_DIM], f32, tag="oo", name="oo")
                nc.vector.tensor_add(oo[:], sg[:], xb_all[:, b, :])
                nc.sync.dma_start(out_d[b], oo[:])

    nc.compile()
    return nc


def _prep_in_maps(x_in, Wq, bq, Wk, bk, Wv, bv, use_bf16=None):
    f8 = ml_dtypes.float8_e4m3
    bf = ml_dtypes.bfloat16

    x_flat = np.asarray(x_in, np.float32).reshape(BS, K)
    # swizzled x^T: [128 p, kt, b], scaled and quantized to fp8
    xq = np.ascontiguousarray(
        x_flat.T.reshape(NKT, 128, BS).transpose(1, 0, 2)
    ).reshape(128, NKT * BS)
    xq = np.clip(xq * SX, -240, 240).astype(f8)

    ws = [np.ascontiguousarray(np.asarray(W, np.float32).T)
          for W in (Wq, Wk, Wv)]
    bs = [np.asarray(b, np.float32).reshape(1, K) * (SX * SW)
          for b in (bq, bk, bv)]
    xtp = np.ascontiguousarray(
        np.asarray(x_in, np.float32).transpose(0, 2, 1))       # (BS, D, W)
    ident = np.eye(128, dtype=np.float32).astype(bf)

    in_maps = []
    for c in range(N_CORES):
        cs = slice(CPC * c, CPC * (c + 1))
        m = {
            "xq": xq,
            "wq": np.clip(ws[0][:, cs] * SW, -240, 240).astype(f8),
            "wk": np.clip(ws[1][:, cs] * SW, -240, 240).astype(f8),
            "wv": np.clip(ws[2][:, cs] * SW, -240, 240).astype(f8),
            "bq": bs[0][:, cs].astype(bf),
            "bk": bs[1][:, cs].astype(bf),
            "bv": bs[2][:, cs].astype(bf),
            "ident": ident,
            "xtp": np.ascontiguousarray(xtp[BPC * c:BPC * (c + 1)]),
        }
        in_maps.append(m)
    return in_maps


def _assemble(results):
    out = np.empty((BS, W_DIM, D), np.float32)
    for c in range(N_CORES):
        o = results[c]["out"]                                   # (BPC, D, W)
        out[BPC * c:BPC * (c + 1)] = o.transpose(0, 2, 1)
    return out


USE_BF16 = True  # kept for timing.py compat; unused


def get_nc(use_bf16=None):
    if "nc" not in _CACHE:
        _CACHE["nc"] = _build()
    return _CACHE["nc"]


def kernel(x_in, Wq, bq, Wk, bk, Wv, bv):
    nc = get_nc()
    in_maps = _prep_in_maps(x_in, Wq, bq, Wk, bk, Wv, bv)
    res = bass_utils.run_bass_kernel_spmd(
        nc, in_maps, core_ids=list(range(N_CORES)))
    return _assemble(res.results)
